# revision 8
# baseline (speedup 1.0000x reference)
"""Trainium2 Bass kernel v2 for nn_AttentionSE3 (graph attention message passing).

Architecture (per core, node-major ELL):
- Host packs nodes into 128-node blocks sorted by degree; each block group g has
  uniform padded degree D_g.  k rows are stored per slot with columns permuted to
  (k16-outer, h8-inner); v rows to (c12-outer, h8-inner); q per node as (k, h).
- Same-D groups are concatenated into chunks so every engine op is wide.
- VectorE: w = k*q (q broadcast over d), radix-2 halving tree over k (all
  contiguous), seg-sum over d, normalize ew by 1/(denom), wv = v*ewn.
- ScalarE: exp activation, PSUM->SBUF output drains.
- TensorE (PE): per-(g,d) transpose of the wv slice [128n x 96] accumulated into
  PSUM -> out_T[(c,h) x n] = the d-reduction.  GpSimd is never used (concurrent
  GpSimd activity slows VectorE ~4x on TRN2).
- Pad slots contribute exp(0)=1 to the denominator; a host pad-count corrects it
  exactly (zero-degree nodes get denom 1 and zero output).
"""

import numpy as np

import concourse.bacc as bacc
import concourse.mybir as mybir
from concourse import tile
from concourse.bass_utils import run_bass_kernel_spmd

try:
    import ml_dtypes
    BF16_NP = np.dtype(ml_dtypes.bfloat16)
except ImportError:  # pragma: no cover
    BF16_NP = None

N_NODES = 50000
H = 8
P = 128
N_CORES = 8
SCALE = float(1.0 / np.sqrt(128.0))
F32 = mybir.dt.float32
BF16 = mybir.dt.bfloat16

MAX_CHUNK_COLS = 2560  # max k-cols per chunk tile (SBUF budget)

# column permutations (relative to the reference layouts)
PERM_K = (np.arange(128).reshape(16, 8).T.reshape(-1))  # want col' = k*8+h; old col = h*16+k
# PERM_K[new] = old: new = k*8+h -> old h*16+k
PERM_K = np.array([ (new % 8) * 16 + (new // 8) for new in range(128)], dtype=np.int64)
PERM_V = np.arange(96).reshape(8, 12).T.reshape(-1)  # new col c*8+h -> old h*12+c
PERM_V_INV = np.argsort(PERM_V)


# ---------------------------------------------------------------- host prep

def prepare(value, key, query0, query1, edge_index, n_nodes=N_NODES, n_cores=N_CORES):
    value = np.asarray(value, dtype=np.float32)
    key = np.asarray(key, dtype=np.float32)
    query0 = np.asarray(query0, dtype=np.float32)
    query1 = np.asarray(query1, dtype=np.float32)
    n_edges = key.shape[0]

    dst = np.asarray(edge_index[1], dtype=np.int64)
    deg = np.bincount(dst, minlength=n_nodes).astype(np.int64)
    n_pad = -(-n_nodes // (P * n_cores)) * (P * n_cores)
    deg_pad = np.concatenate([deg, np.zeros(n_pad - n_nodes, dtype=np.int64)])
    nb = n_pad // P
    ng = nb // n_cores

    order = np.argsort(deg_pad, kind="stable")
    degs_o = deg_pad[order]

    blk_max = degs_o.reshape(nb, P).max(axis=1)
    D_eff = np.maximum(blk_max.reshape(ng, n_cores).max(axis=1), 1).astype(np.int64)
    off = np.concatenate([[0], np.cumsum(P * D_eff)]).astype(np.int64)
    S = int(off[-1])

    pos = np.arange(n_pad)
    block = pos // P
    g_of = block // n_cores
    core_of = block % n_cores
    row = pos % P
    Dg = D_eff[g_of]
    base = off[g_of] + row * Dg

    edge_order = np.argsort(dst, kind="stable")
    starts = np.concatenate([[0], np.cumsum(deg)])

    pp = np.repeat(pos, degs_o)
    cum0 = np.concatenate([[0], np.cumsum(degs_o)])[:-1]
    d_idx = np.arange(n_edges) - np.repeat(cum0, degs_o)
    node_of_pp = order[pp]
    edge_ids = edge_order[starts[node_of_pp] + d_idx]
    slot_global = core_of[pp] * S + base[pp] + d_idx

    kp = np.zeros((n_cores * S, 128), dtype=np.float32)
    kp[slot_global] = key[:, PERM_K][edge_ids]
    vp = np.zeros((n_cores * S, 96), dtype=np.float32)
    vp[slot_global] = value.reshape(n_edges, 96)[:, PERM_V][edge_ids]
    kp = kp.reshape(n_cores, S, 128)
    vp = vp.reshape(n_cores, S, 96)
    # interleave k and v into one stream: per group, per node, the row content is
    # [k-block (k16, d, h8) D*128][v-block (d, c12, h8) D*96], carved into D rows
    # of 224 so one DMA per group brings both with large contiguous packets
    ng_local = len(D_eff)
    kv = np.zeros((n_cores, S, 224), dtype=np.float32)
    for g in range(ng_local):
        D = int(D_eff[g]); s0 = int(off[g]); s1 = int(off[g + 1])
        kblk = (kp[:, s0:s1, :].reshape(n_cores, P, D, 16, 8)
                .transpose(0, 1, 3, 2, 4).reshape(n_cores, P, D * 128))
        vblk = vp[:, s0:s1, :].reshape(n_cores, P, D * 96)
        kv[:, s0:s1, :] = np.concatenate([kblk, vblk], axis=2).reshape(
            n_cores, P * D, 224)

    # q per node, cols (k-outer, h-inner) to match k layout
    qfull = np.concatenate([query0, query1], axis=-1).reshape(n_nodes, 128)[:, PERM_K]
    q_pad = np.zeros((n_pad, 128), dtype=np.float32)
    q_pad[:n_nodes] = qfull
    q_sorted = q_pad[order].reshape(nb, P, 128)

    pc = (Dg - degs_o).astype(np.float32)
    zero_deg = degs_o == 0
    pc[zero_deg] = (Dg[zero_deg] - 1).astype(np.float32)
    pc_sorted = pc.reshape(nb, P)

    dt = BF16_NP
    kv = kv.astype(dt)
    in_maps = []
    ident = np.eye(P, dtype=np.float32).astype(dt)
    for c in range(n_cores):
        q_c = np.ascontiguousarray(
            q_sorted[c::n_cores].transpose(1, 0, 2).reshape(P, ng * 128)).astype(dt)
        # pad counts pre-expanded over heads: [128, ng*8] f32
        pc_c = np.repeat(np.ascontiguousarray(pc_sorted[c::n_cores].T), H, axis=1)
        in_maps.append({"kv": kv[c], "q": q_c, "pc": pc_c, "ident": ident})

    meta = dict(D_eff=D_eff, off=off, S=S, NG=ng, NB=nb, order=order,
                n_nodes=n_nodes, n_pad=n_pad)
    return in_maps, meta


def unshard_output(out_cores, meta):
    """out_cores: list of [128(c,h), NG*128(n)] -> [n_nodes, 32, 3]."""
    ng, nb = meta["NG"], meta["NB"]
    n_cores = len(out_cores)
    order, n_nodes, n_pad = meta["order"], meta["n_nodes"], meta["n_pad"]
    out_sorted = np.zeros((nb, P, 96), dtype=np.float32)
    for c in range(n_cores):
        # out2[p=(c,h), g*128+n] -> [g, n, 96]
        oc = np.asarray(out_cores[c], dtype=np.float32)[:96]  # [96, ng*128]
        out_sorted[c::n_cores] = oc.reshape(96, ng, P).transpose(1, 2, 0)
    out_sorted = out_sorted.reshape(n_pad, 96)[:, PERM_V_INV]
    out_full = np.zeros((n_nodes, 96), dtype=np.float32)
    mask = order < n_nodes
    out_full[order[mask]] = out_sorted[mask]
    return out_full.reshape(n_nodes, 32, 3)


# ---------------------------------------------------------------- bass kernel

def make_chunks(D_eff):
    """Greedy: consecutive same-D groups, k-cols <= MAX_CHUNK_COLS.
    Emitted largest-D first so the pipeline has long ops while the
    PE/Scalar ew chains fill."""
    chunks = []  # (g_start, n_groups, D)
    g = 0
    ng = len(D_eff)
    while g < ng:
        D = int(D_eff[g])
        n = 1
        while (g + n < ng and int(D_eff[g + n]) == D
               and (n + 1) * D * 128 <= MAX_CHUNK_COLS):
            n += 1
        chunks.append((g, n, D))
        g += n
    chunks.sort(key=lambda c: -c[2])
    smallest = min(range(len(chunks)), key=lambda i: chunks[i][1] * chunks[i][2])
    chunks.insert(0, chunks.pop(smallest))
    return chunks


def build(D_eff, S, NG, n_cores=N_CORES):
    D_eff = [int(d) for d in D_eff]
    off = np.concatenate([[0], np.cumsum([P * d for d in D_eff])]).astype(np.int64)
    chunks = make_chunks(D_eff)

    nc = bacc.Bacc("TRN2", target_bir_lowering=False, debug=False,
                   num_devices=n_cores)
    kvp = nc.declare_dram_parameter("kv", [S, 224], BF16, isOutput=False)
    q = nc.declare_dram_parameter("q", [P, NG * 128], BF16, isOutput=False)
    pc = nc.declare_dram_parameter("pc", [P, NG * H], F32, isOutput=False)
    id_in = nc.declare_dram_parameter("ident", [P, P], BF16, isOutput=False)
    out = nc.declare_dram_parameter("out", [P, NG * 128], BF16, isOutput=True)

    mult = mybir.AluOpType.mult
    add = mybir.AluOpType.add
    sub = mybir.AluOpType.subtract
    AX = mybir.AxisListType.X

    with tile.TileContext(nc) as tc:
        with tc.tile_pool(name="res", bufs=1) as res, \
             tc.tile_pool(name="work", bufs=2) as work, \
             tc.tile_pool(name="small", bufs=2) as small, \
             tc.psum_pool(name="ps", bufs=2) as ps:
            QSPLIT = chunks[0][0] + chunks[0][1]
            qa = res.tile([P, QSPLIT * 128], BF16)
            qb_t = res.tile([P, (NG - QSPLIT) * 128], BF16)
            pc_sb = res.tile([P, NG * H], F32)
            ident = res.tile([P, P], BF16)
            nc.sync.dma_start(ident[:], id_in[:])
            HALF_G = NG // 2
            out2a = res.tile([P, HALF_G * 128], BF16)
            out2b = res.tile([P, (NG - HALF_G) * 128], BF16)

            state = {}
            dstate = {}

            def emit_dma(ci):
                (g0, G, D) = chunks[ci]
                s0 = int(off[g0])
                Lk = G * D * 128
                Lv = G * D * 96
                kv = work.tile([P, G * D * 224], BF16, tag="kv", bufs=7)
                for gl in range(G):
                    sg = s0 + gl * P * D
                    nc.sync.dma_start(
                        kv[:, gl * D * 224:(gl + 1) * D * 224],
                        kvp[sg:sg + P * D, :].rearrange("(n d) f -> n (d f)", n=P))
                dstate[ci] = (kv,)

            def emit_p1(ci):
                (g0, G, D) = chunks[ci]
                s0 = int(off[g0])
                Lk = G * D * 128
                Lv = G * D * 96
                GD = G * D
                (kv,) = dstate.pop(ci)
                kv3 = kv[:].rearrange("n (g x) -> n g x", g=G)
                ktv = kv3[:, :, :D * 128]
                vtv = kv3[:, :, D * 128:]

                # w = k * q (q broadcast over d) : [P, G, 16k, D, 8h]
                if g0 + G <= QSPLIT:
                    qsrc = qa[:, g0 * 128:(g0 + G) * 128]
                else:
                    qsrc = qb_t[:, (g0 - QSPLIT) * 128:(g0 + G - QSPLIT) * 128]
                qb = (qsrc
                      .rearrange("n (g k h) -> n g k h", g=G, k=16)
                      .unsqueeze(3).broadcast_to([P, G, 16, D, 8]))
                w = work.tile([P, Lk], BF16, tag="w", bufs=3)
                nc.vector.tensor_tensor(
                    out=w[:].rearrange("n (g k d h) -> n g k d h", g=G, k=16, d=D),
                    in0=ktv.rearrange("n g (k d h) -> n g k d h", k=16, d=D),
                    in1=qb, op=mult)

                ew = small.tile([P, GD * 8], BF16, tag="ew", bufs=9)
                if D <= 36:
                    # PE k-reduction over d-chunks of <=16 (psum partitions <=128)
                    # phase A: all transpose-accumulate chains + exps;
                    # phase B: all transpose-backs + drains (keeps PE stream
                    # from stalling on the Scalar exp roundtrip per group)
                    ewTs = []
                    for gl in range(G):
                        base = gl * D * 128
                        for d0 in range(0, D, 16):
                            dw = min(16, D - d0)
                            plg = ps.tile([dw * 8, P], F32, tag="plg", bufs=3)
                            for k in range(16):
                                sl = base + k * D * 8 + d0 * 8
                                nc.tensor.matmul(
                                    plg[:], lhsT=w[:, sl:sl + dw * 8], rhs=ident[:],
                                    start=(k == 0), stop=(k == 15))
                            ewT = small.tile([dw * 8, P], BF16, tag="ewT", bufs=4)
                            nc.scalar.activation(
                                out=ewT[:], in_=plg[:],
                                func=mybir.ActivationFunctionType.Exp, scale=SCALE)
                            ewTs.append((gl, d0, dw, ewT))
                    for (gl, d0, dw, ewT) in ewTs:
                        pew = ps.tile([P, dw * 8], F32, tag="pew", bufs=2)
                        nc.tensor.matmul(
                            pew[:], lhsT=ewT[:], rhs=ident[:dw * 8, :dw * 8],
                            start=True, stop=True)
                        nc.scalar.activation(
                            out=ew[:, gl * D * 8 + d0 * 8:
                                    gl * D * 8 + (d0 + dw) * 8],
                            in_=pew[:],
                            func=mybir.ActivationFunctionType.Copy)
                else:
                    # radix-2 halving tree over k (k-outer layout: contiguous)
                    t8 = small.tile([P, GD * 64], BF16, tag="t8")
                    w3 = w[:].rearrange("n (g k c) -> n g k c", g=G, k=16)
                    nc.vector.tensor_tensor(
                        out=t8[:].rearrange("n (g k c) -> n g k c", g=G, k=8),
                        in0=w3[:, :, :8], in1=w3[:, :, 8:], op=add)
                    t4 = small.tile([P, GD * 32], BF16, tag="t4")
                    t83 = t8[:].rearrange("n (g k c) -> n g k c", g=G, k=8)
                    nc.vector.tensor_tensor(
                        out=t4[:].rearrange("n (g k c) -> n g k c", g=G, k=4),
                        in0=t83[:, :, :4], in1=t83[:, :, 4:], op=add)
                    t2 = small.tile([P, GD * 16], BF16, tag="t2")
                    t43 = t4[:].rearrange("n (g k c) -> n g k c", g=G, k=4)
                    nc.vector.tensor_tensor(
                        out=t2[:].rearrange("n (g k c) -> n g k c", g=G, k=2),
                        in0=t43[:, :, :2], in1=t43[:, :, 2:], op=add)
                    lg = small.tile([P, GD * 8], BF16, tag="lg")
                    t23 = t2[:].rearrange("n (g k c) -> n g k c", g=G, k=2)
                    nc.vector.tensor_tensor(
                        out=lg[:].rearrange("n (g k c) -> n g k c", g=G, k=1),
                        in0=t23[:, :, :1], in1=t23[:, :, 1:], op=add)
                    nc.scalar.activation(out=ew[:], in_=lg[:],
                                         func=mybir.ActivationFunctionType.Exp,
                                         scale=SCALE)

                state[ci] = (ew, vtv, G, D, g0)

            def emit_p23(ci):
                (ew, vtv, G, D, g0) = state.pop(ci)
                GD = G * D
                Lv = G * D * 96
                # seg-sum over d -> denominators [P, G*8] f32
                seg = small.tile([P, G * 8], F32, tag="seg")
                nc.vector.tensor_reduce(
                    out=seg[:],
                    in_=ew[:].rearrange("n (g d h) -> n g h d", g=G, d=D),
                    axis=AX, op=add)
                # denom = seg - pad_count ; r = 1/denom
                dn = small.tile([P, G * 8], F32, tag="dn")
                nc.vector.tensor_tensor(
                    out=dn[:], in0=seg[:],
                    in1=pc_sb[:, g0 * H:(g0 + G) * H], op=sub)
                r = small.tile([P, G * 8], F32, tag="r")
                nc.vector.reciprocal(out=r[:], in_=dn[:])

                # ewn = ew * r (r broadcast over d), bf16
                ewn = small.tile([P, GD * 8], BF16, tag="ewn", bufs=2)
                rb = (r[:].rearrange("n (g h) -> n g h", g=G)
                      .unsqueeze(2).broadcast_to([P, G, D, 8]))
                nc.vector.tensor_tensor(
                    out=ewn[:].rearrange("n (g d h) -> n g d h", g=G, d=D),
                    in0=ew[:].rearrange("n (g d h) -> n g d h", g=G, d=D),
                    in1=rb, op=mult)

                # wv = v * ewn (ewn broadcast over c=12)
                wv = work.tile([P, Lv], BF16, tag="wv")
                eb = (ewn[:].rearrange("n (g d h) -> n g d h", g=G, d=D)
                      .unsqueeze(3).broadcast_to([P, G, D, 12, 8]))
                nc.vector.tensor_tensor(
                    out=wv[:].rearrange("n (g d c h) -> n g d c h", g=G, d=D, c=12),
                    in0=vtv.rearrange("n g (d c h) -> n g d c h", d=D, c=12),
                    in1=eb, op=mult)

                # PE: out_T[g] = sum_d transpose(wv[:, g, d, :, :])  [96 x 128n]
                for gl in range(G):
                    pout = ps.tile([96, P], F32, tag="pout")
                    for d in range(D):
                        sl = (gl * D + d) * 96
                        nc.tensor.matmul(
                            pout[:], lhsT=wv[:, sl:sl + 96], rhs=ident[:],
                            start=(d == 0), stop=(d == D - 1))
                    # drain psum -> out2 half (ScalarE copy, bf16)
                    gg = g0 + gl
                    if gg < HALF_G:
                        odst = out2a[:96, gg * 128:(gg + 1) * 128]
                    else:
                        odst = out2b[:96, (gg - HALF_G) * 128:(gg - HALF_G + 1) * 128]
                    nc.scalar.activation(
                        out=odst, in_=pout[:],
                        func=mybir.ActivationFunctionType.Copy)

            DEPTH = 6
            NC = len(chunks)
            for ci in range(NC):
                if ci == 0:
                    emit_dma(0)
                    nc.sync.dma_start(qa[:], q[:, :QSPLIT * 128])
                    nc.sync.dma_start(qb_t[:], q[:, QSPLIT * 128:])
                    emit_dma(1)
                    nc.sync.dma_start(pc_sb[:], pc[:])
                elif ci + 1 < NC:
                    emit_dma(ci + 1)
                emit_p1(ci)
                if ci >= DEPTH:
                    emit_p23(ci - DEPTH)
            for ci in range(max(0, NC - DEPTH), NC):
                emit_p23(ci)

            nc.sync.dma_start(out[:, :HALF_G * 128], out2a[:])
            nc.sync.dma_start(out[:, HALF_G * 128:], out2b[:])

    nc.compile()
    return nc


# ---------------------------------------------------------------- entry point

LAST_RESULT = None


def kernel(value, key, query0, query1, edge_index):
    global LAST_RESULT
    import os
    in_maps, meta = prepare(value, key, query0, query1, edge_index)
    nc = build(meta["D_eff"], meta["S"], meta["NG"])
    res = run_bass_kernel_spmd(nc, in_maps, list(range(N_CORES)),
                               tmpdir=os.environ.get("BASS_SPMD_TMPDIR"))
    LAST_RESULT = res
    out_cores = [res.results[c]["out"] for c in range(N_CORES)]
    return unshard_output(out_cores, meta)
